# revision 1
# baseline (speedup 1.0000x reference)
"""Trainium2 Bass kernel for BodyStructureLoss.

Computes: mean over (B, J) of where(||kps[b,j,:]|| > 1.0, ||kps[b,j,:]||, 0)
for kps of shape [524288, 17, 3] float32.

Strategy (data-parallel over 8 NeuronCores):
  - Each core gets B/8 = 65536 batch rows = 3,342,336 contiguous floats,
    viewed as [128 partitions, 26112] (each partition row holds 8704
    complete (x,y,z) triplets).
  - Tiles of F columns stream in via DMA; squares run in-place (ACT, or DVE
    for a few tiles to balance engines); DVE sums the 3 squared components
    with two strided adds into a shared per-pair s tile; per tile PAIR one
    ACT sqrt and two DVE tensor_scalar+accumulate ops produce
    sum(max(d,1)) and count(s>1) columns.
  - Per core the [128, 2*n_pairs] accumulator tile is DMA'd out directly;
    the host sums all partials across cores and applies
    masked_sum = sum(max(d,1)) + count - B*J, then divides by B*J.
"""

import os

import numpy as np

# the NTFF trace path needs antenv.axon_hooks, which this client image lacks;
# force-disable so a stray BASS_TRACE=1 in the environment cannot break runs
os.environ["BASS_NEVER_TRACE"] = "1"

import concourse.bacc as bacc
import concourse.mybir as mybir
from concourse.bass_utils import run_bass_kernel_spmd
from concourse.tile import TileContext

B, J, D = 524288, 17, 3
HALF_BODY = 1.0  # threshold/2 with threshold=2.0
N_CORES = 8
B_SHARD = B // N_CORES  # 65536
P = 128
FLOATS_PER_CORE = B_SHARD * J * D  # 3342336
COLS = FLOATS_PER_CORE // P  # 26112 (divisible by 3: 26112 = 3*8704)

_DT = mybir.dt.float32

# default plan: pairs of tile column-counts; each pair shares one sqrt.
# small first pair ramps the compute pipeline early; small tail pairs
# shorten the post-DMA drain.
PLAN = [[408, 408]] + [[1632, 1632]] * 7 + [[816, 816], [408, 408]]
DVE_SQ = frozenset({1})  # tile indices squared on DVE instead of ACT


def build_nc(P=P, COLS=COLS, plan=None, dve_sq=DVE_SQ, pipelined=True, repeat=1, lag=1, flush_from=None, split_sq=frozenset(), pool_mode="stack", flush_before=None):
    if plan is None:
        plan = PLAN
    flat = [f for pair in plan for f in pair]
    assert sum(flat) == COLS
    assert all(f % 3 == 0 for f in flat)
    n_pairs = len(plan)
    M_MAX = max(sum(pair) for pair in plan) // 3

    nc = bacc.Bacc(
        "TRN2", target_bir_lowering=False, debug=False, num_devices=N_CORES
    )
    x = nc.dram_tensor("x", [P, COLS], _DT, kind="ExternalInput")
    out = nc.dram_tensor(
        "out", [P, 2 * len(plan) * repeat], _DT, kind="ExternalOutput"
    )

    with TileContext(nc, pool_alloc_mode=pool_mode) as tc:
        with (
            tc.tile_pool(name="xin", bufs=4) as xpool,
            tc.tile_pool(name="small", bufs=4) as spool,
            tc.tile_pool(name="accp", bufs=1) as accpool,
        ):
            # two accumulator columns per pair: sum(max(d,1)) and count(s>1)
            accs = accpool.tile([P, 2 * n_pairs * repeat], _DT)
            # shared scratch for tensor_scalar main outputs (only accum_out
            # is consumed); WAW chains are DVE-internal and in-order
            scr32 = accpool.tile([P, M_MAX], _DT)
            scr16 = accpool.tile([P, M_MAX], mybir.dt.bfloat16)

            # dummy sqrt first: makes bacc's table pass load sqrt_and_others
            # (which also contains Square), avoiding a second ACT table load
            nc.vector.memset(scr32[:, :1], 1.0)
            nc.scalar.activation(
                out=scr32[:, :1],
                in_=scr32[:, :1],
                func=mybir.ActivationFunctionType.Sqrt,
            )

            # stage A (per tile): DMA -> square in place -> adds into s slice
            def stage_a(gi, col0, F, s2, s_off):
                M = F // 3
                sz = str(F)
                xt = xpool.tile([P, F], _DT, tag="xt" + sz)
                nc.sync.dma_start(out=xt, in_=x[:, col0 : col0 + F])

                if gi in dve_sq:
                    nc.vector.tensor_tensor(
                        out=xt, in0=xt, in1=xt, op=mybir.AluOpType.mult
                    )
                elif gi in split_sq:
                    # fractional engine split: ACT squares the front 2/3,
                    # DVE the back 1/3 of this tile
                    c = (2 * F // 3) & ~3
                    nc.scalar.activation(
                        out=xt[:, :c], in_=xt[:, :c],
                        func=mybir.ActivationFunctionType.Square,
                    )
                    nc.vector.tensor_tensor(
                        out=xt[:, c:], in0=xt[:, c:], in1=xt[:, c:],
                        op=mybir.AluOpType.mult,
                    )
                else:
                    nc.scalar.activation(
                        out=xt, in_=xt, func=mybir.ActivationFunctionType.Square
                    )
                sq3 = xt.rearrange("p (m t) -> p m t", t=3)
                sl = s2[:, s_off : s_off + M]
                nc.vector.tensor_tensor(
                    out=sl, in0=sq3[:, :, 0], in1=sq3[:, :, 1], op=mybir.AluOpType.add
                )
                nc.vector.tensor_tensor(
                    out=sl, in0=sl, in1=sq3[:, :, 2], op=mybir.AluOpType.add
                )

            # stage B (per pair): DVE count(s>1) | ACT sqrt -> DVE sum max(d,1)
            def stage_b(pi, s2, M2):
                sz = str(M2)
                # count(s > 1): only depends on s, runs while ACT sqrts
                nc.vector.tensor_scalar(
                    out=scr32[:, :M2],
                    in0=s2,
                    scalar1=float(HALF_BODY * HALF_BODY),
                    scalar2=None,
                    op0=mybir.AluOpType.is_gt,
                    op1=mybir.AluOpType.add,
                    accum_out=accs[:, 2 * pi + 1 : 2 * pi + 2],
                )
                # d in bf16: mask precision comes from fp32 s (count term);
                # max(d,1)+count is continuous in d, so bf16 rounding of d
                # contributes only ~1e-3 relative noise per element that
                # cancels in the sum. bf16 d makes this TS 4x mode.
                d = spool.tile([P, M2], mybir.dt.bfloat16, tag="d" + sz)
                nc.scalar.activation(
                    out=d, in_=s2, func=mybir.ActivationFunctionType.Sqrt
                )
                # sum(max(d, 1)) = sum(relu(d-1)) + M2 per partition
                nc.vector.tensor_scalar(
                    out=scr16[:, :M2],
                    in0=d,
                    scalar1=float(HALF_BODY),
                    scalar2=None,
                    op0=mybir.AluOpType.max,
                    op1=mybir.AluOpType.add,
                    accum_out=accs[:, 2 * pi : 2 * pi + 1],
                )

            # emit: stage_a per tile; stage_b lags `lag` pairs behind.
            # repeat>1 re-runs the whole pass (benchmarking only).
            from collections import deque

            pending = deque()  # (pair_idx, s2, M2)
            for r in range(repeat):
                col0 = 0
                gi = 0
                for pi0, pair in enumerate(plan):
                    pi = r * n_pairs + pi0
                    # emit pending stage_b BEFORE this pair's stage_a, so a
                    # ready sqrt is not queued behind a DMA-gated square
                    if flush_before is not None and pi0 >= flush_before:
                        while pending:
                            stage_b(*pending.popleft())
                    M2 = sum(pair) // 3
                    s2 = spool.tile([P, M2], _DT, tag="s" + str(M2))
                    s_off = 0
                    for F in pair:
                        stage_a(gi, col0, F, s2, s_off)
                        col0 += F
                        s_off += F // 3
                        gi += 1
                    if pipelined:
                        pending.append((pi, s2, M2))
                        # from pair `flush_from` on, emit stage_b immediately
                        # so tail sqrts outrank later squares in the scheduler
                        eff_lag = 0 if (flush_from is not None and pi0 >= flush_from) else lag
                        while len(pending) > eff_lag:
                            stage_b(*pending.popleft())
                    else:
                        stage_b(pi, s2, M2)
            while pending:
                stage_b(*pending.popleft())

            nc.sync.dma_start(out=out[:, :], in_=accs)

    nc.compile()
    return nc


_nc_cache = None
last_results = None


def kernel(kps_world_pred: np.ndarray) -> np.ndarray:
    global _nc_cache, last_results
    x = np.ascontiguousarray(kps_world_pred, dtype=np.float32)
    assert x.shape == (B, J, D)

    shards = x.reshape(N_CORES, P, COLS)
    in_maps = [{"x": shards[c]} for c in range(N_CORES)]

    if _nc_cache is None:
        _nc_cache = build_nc()

    # the axon terminal occasionally reports a transient
    # NRT_EXEC_UNIT_UNRECOVERABLE left over from a previous run; it clears
    # after a short wait, so retry rather than fail the whole call
    import time

    res = None
    for attempt in range(3):
        try:
            res = run_bass_kernel_spmd(_nc_cache, in_maps, list(range(N_CORES)))
            break
        except Exception:
            if attempt == 2:
                raise
            time.sleep(15)
    last_results = res

    # per-partition device partials hold sum(max(d,1)) + count(s>1)
    #   = masked_sum + n_triplets, so subtract the global triplet count.
    total = np.float64(0.0)
    for c in range(N_CORES):
        total += res.results[c]["out"].astype(np.float64).sum()
    total -= np.float64(B * J)
    return np.asarray(total / (B * J), dtype=np.float32)



# revision 3
# speedup vs baseline: 1.0155x; 1.0155x over previous
"""Trainium2 Bass kernel for BodyStructureLoss.

Computes: mean over (B, J) of where(||kps[b,j,:]|| > 1.0, ||kps[b,j,:]||, 0)
for kps of shape [524288, 17, 3] float32.

Strategy (data-parallel over 8 NeuronCores):
  - Each core gets B/8 = 65536 batch rows viewed as [128, 26112] fp32.
  - Tiles of F columns stream in via DMA. Each tile is squared with a
    transposed write into a bf16 tile yt so the three components of each
    (x,y,z) triplet land in separate contiguous thirds:
        yt[:, t*M + m] = xt[:, 3m + t]^2   (M = F/3)
    The squaring engine alternates ACT (activation Square) / DVE
    (tensor_tensor mult) to balance load.
  - Two packed bf16 tensor_tensor adds (DVE 2x mode) produce the
    per-triplet squared norm s [P, M].
  - Per tile: tensor_scalar is_gt (4x bf16) accumulates count(s > 1) into
    accs[:, 2i+1]; ACT sqrt gives d bf16; tensor_scalar max (4x bf16)
    accumulates sum(max(d, 1)) into accs[:, 2i]. Counts run on the GPSIMD
    (Pool) engine mid-stream to keep DVE free.
  - Host sums all partials: sum(accs) = masked_sum + B*J, so subtract
    B*J and divide by B*J.
"""

import os

import numpy as np

# the NTFF trace path needs antenv.axon_hooks, which this client image lacks;
# force-disable so a stray BASS_TRACE=1 in the environment cannot break runs
os.environ["BASS_NEVER_TRACE"] = "1"

import concourse.bacc as bacc
import concourse.mybir as mybir
from concourse.bass_utils import run_bass_kernel_spmd
from concourse.tile import TileContext

B, J, D = 524288, 17, 3
HALF_BODY = 1.0  # threshold/2 with threshold=2.0
N_CORES = 8
P = 128
COLS = (B // N_CORES) * J * D // P  # 26112

_DT = mybir.dt.float32
_BF = mybir.dt.bfloat16


def _default_cfg():
    ramp = [612, 1020]
    tail = [1224, 1020, 612, 408]
    body = COLS - sum(ramp) - sum(tail)
    assert body % 1632 == 0
    tiles = ramp + [1632] * (body // 1632) + tail
    n = len(tiles)
    nt = len(tail)
    sq = {i: ("A" if i % 2 == 0 else "V") for i in range(n - nt)}
    for k, e in enumerate("VAVA"):
        sq[n - nt + k] = e
    return {"tiles": tiles, "sq_eng": sq, "b_lag": 2, "sbufs": 8}


def build_nc(cfg=None):
    if cfg is None:
        cfg = _default_cfg()
    tiles = cfg["tiles"]
    assert sum(tiles) == COLS
    assert all(f % 3 == 0 for f in tiles)
    n_t = len(tiles)
    M_MAX = max(tiles) // 3
    F_PAD = max(tiles)
    sq_eng = cfg.get("sq_eng", {})
    add_eng = cfg.get("add_eng", {})
    count_eng = cfg.get("count_eng", {})
    b_lag = cfg.get("b_lag", 1)
    flush_from = cfg.get("flush_from", None)

    nc = bacc.Bacc(
        "TRN2", target_bir_lowering=False, debug=False, num_devices=N_CORES
    )
    x = nc.dram_tensor("x", [P, COLS], _DT, kind="ExternalInput")
    out = nc.dram_tensor("out", [P, 2 * n_t], _DT, kind="ExternalOutput")

    with TileContext(nc, pool_alloc_mode=cfg.get("pool_mode", "stack")) as tc:
        with (
            tc.tile_pool(name="xin", bufs=cfg.get("xbufs", 6)) as xpool,
            tc.tile_pool(name="ysq", bufs=cfg.get("ybufs", 4)) as ypool,
            tc.tile_pool(name="small", bufs=cfg.get("sbufs", 4)) as spool,
            tc.tile_pool(name="accp", bufs=1) as accpool,
        ):
            accs = accpool.tile([P, 2 * n_t], _DT)
            scrA = accpool.tile([P, M_MAX], _BF)
            scrB = accpool.tile([P, M_MAX], _BF)

            # dummy sqrt makes bacc's table pass load sqrt_and_others (which
            # also contains Square), avoiding a mid-stream ACT table load
            nc.vector.memset(scrA[:, :2], 1.0)
            nc.scalar.activation(
                out=scrA[:, :2],
                in_=scrA[:, :2],
                func=mybir.ActivationFunctionType.Sqrt,
            )

            def stage_a(i, col0, F):
                M = F // 3
                xt_full = xpool.tile([P, F_PAD], _DT, tag="xt")
                xt = xt_full[:, :F]
                nc.sync.dma_start(out=xt, in_=x[:, col0 : col0 + F])
                yt_full = ypool.tile([P, F_PAD], _BF, tag="yt")
                yt = yt_full[:, :F]
                # transposed write view: yv iterated (m, t) -> offset t*M + m
                yv = yt.rearrange("p (t m) -> p m t", t=3)
                se = sq_eng.get(i, "A")
                if se == "V":
                    nc.vector.tensor_tensor(
                        out=yv, in0=xt, in1=xt, op=mybir.AluOpType.mult
                    )
                elif se == "A":
                    nc.scalar.activation(
                        out=yv, in_=xt, func=mybir.ActivationFunctionType.Square
                    )
                else:  # float: ACT front fraction, DVE the rest
                    c = 3 * (int(F * se) // 12) * 4
                    nc.scalar.activation(
                        out=yv[:, : c // 3, :], in_=xt[:, :c],
                        func=mybir.ActivationFunctionType.Square,
                    )
                    nc.vector.tensor_tensor(
                        out=yv[:, c // 3 :, :], in0=xt[:, c:], in1=xt[:, c:],
                        op=mybir.AluOpType.mult,
                    )
                s_full = spool.tile([P, M_MAX], _BF, tag="s")
                s = s_full[:, :M]
                nc.vector.tensor_tensor(
                    out=s, in0=yt[:, :M], in1=yt[:, M : 2 * M],
                    op=mybir.AluOpType.add,
                )
                nc.vector.tensor_tensor(
                    out=s, in0=s, in1=yt[:, 2 * M : 3 * M],
                    op=mybir.AluOpType.add,
                )
                return s, M

            def stage_b(i, s, M):
                nc.vector.tensor_scalar(
                    out=scrA[:, :M],
                    in0=s,
                    scalar1=float(HALF_BODY * HALF_BODY),
                    scalar2=None,
                    op0=mybir.AluOpType.is_gt,
                    op1=mybir.AluOpType.add,
                    accum_out=accs[:, 2 * i + 1 : 2 * i + 2],
                )
                d_full = spool.tile([P, M_MAX], _BF, tag="d")
                d = d_full[:, :M]
                nc.scalar.activation(
                    out=d, in_=s, func=mybir.ActivationFunctionType.Sqrt
                )
                nc.vector.tensor_scalar(
                    out=scrB[:, :M],
                    in0=d,
                    scalar1=float(HALF_BODY),
                    scalar2=None,
                    op0=mybir.AluOpType.max,
                    op1=mybir.AluOpType.add,
                    accum_out=accs[:, 2 * i : 2 * i + 1],
                )

            from collections import deque

            pending = deque()
            col0 = 0
            for i, F in enumerate(tiles):
                s, M = stage_a(i, col0, F)
                col0 += F
                pending.append((i, s, M))
                eff_lag = 0 if (flush_from is not None and i >= flush_from) else b_lag
                while len(pending) > eff_lag:
                    stage_b(*pending.popleft())
            while pending:
                stage_b(*pending.popleft())

            nc.sync.dma_start(out=out[:, :], in_=accs)

    nc.compile()
    return nc


_nc_cache = None
last_results = None


def kernel(kps_world_pred: np.ndarray) -> np.ndarray:
    global _nc_cache, last_results
    x = np.ascontiguousarray(kps_world_pred, dtype=np.float32)
    assert x.shape == (B, J, D)

    shards = x.reshape(N_CORES, P, COLS)
    in_maps = [{"x": shards[c]} for c in range(N_CORES)]

    if _nc_cache is None:
        _nc_cache = build_nc()

    # the axon terminal occasionally reports a transient
    # NRT_EXEC_UNIT_UNRECOVERABLE left over from a previous run; it clears
    # after a short wait, so retry rather than fail the whole call
    import time

    res = None
    for attempt in range(3):
        try:
            res = run_bass_kernel_spmd(_nc_cache, in_maps, list(range(N_CORES)))
            break
        except Exception:
            if attempt == 2:
                raise
            time.sleep(15)
    last_results = res

    # per-partition device partials hold sum(max(d,1)) + count(s>1)
    #   = masked_sum + n_triplets, so subtract the global triplet count.
    total = np.float64(0.0)
    for c in range(N_CORES):
        total += res.results[c]["out"].astype(np.float64).sum()
    total -= np.float64(B * J)
    return np.asarray(total / (B * J), dtype=np.float32)
